# revision 41
# baseline (speedup 1.0000x reference)
"""Trainium2 Bass kernel for quantized (BitNet-style) multi-head attention.

Model (per batch element): bitlinear(qkv) -> 6-head softmax attention -> bitlinear(proj)
with B=8, N=2048, C=384, H=6, D=64.

Sharding: pure data parallel - one batch element per NeuronCore (8 cores),
weights replicated, no collectives.

The kernel is ACT(exp)-bound: 192 softmax exps of [128,1024] = ~220us of
ScalarE at 1 elem/lane/cycle, so the schedule keeps ScalarE ~100% busy on exp
and hides everything else in the other engines' slack:
  * preamble spine = only what gates the first exp: x DMA'd first and
    quantized in 4 chunks (DVE STT mult+magic / TS sub+clip, PE transposes,
    ACT psum->sbuf copies), qkv-weight quant for the tiles q0/k0/v only,
    q0/k0 for all chunks + v[0:4].  First exp fires at ~65us.
  * everything else (v[4:16], q/k tiles for heads 1-5, proj-weight quant) is
    emitted as "elastic" closures pumped inside the attention loop, ordered
    by when attention will need them (all engine queues are in-order, so
    emission order = schedule; a stalled op blocks its whole queue).
  * per-(head,chunk) softmax-normalize transposes (drains) run in leftover
    iterations; the proj input quant for token chunks 0-6 runs during the
    last head; the proj matmuls run post-loop, software-pipelined
    (transposes/copies of chunk j overlap matmuls of chunk j-1).
  * scale+bias applications are DVE scalar_tensor_tensor folds; the pso->
    stage copy is split DVE/ACT so the next half's first A@V only waits half
    a copy.  gpsimd is used only for memset/partition_broadcast/all_reduce
    (its tensor_scalar is ~20x slower than DVE and starves DVE via the
    shared SBUF port).
"""

import sys
import os

sys.path.insert(0, "/opt/trn_rl_repo")

import numpy as np

import concourse.bass as bass
import concourse.mybir as mybir
import concourse.tile as tile
import concourse.bacc as bacc
from concourse import bass_utils
from concourse.bass_isa import ReduceOp
from concourse.masks import make_identity

F32 = mybir.dt.float32
F16 = mybir.dt.float16
AF = mybir.ActivationFunctionType
ALU = mybir.AluOpType

B, N, C = 8, 2048, 384
H, D = 6, 64
O3 = 3 * C  # 1152
NT = N // 128   # 16 token tiles
CB = C // 128   # 3 contraction blocks
OT = O3 // 128  # 9 qkv output row tiles
NCH = 4         # x chunks
TPC = NT // NCH  # 4 token tiles per chunk
CW = TPC * 128   # 512 tokens per chunk
HN = N // 2      # 1024
MAGIC = float(1.5 * 2**23)  # fp32 round-to-nearest-even via add/sub


def build_program():
    nc = bacc.Bacc("TRN2", target_bir_lowering=False, debug=False, num_devices=8)

    x_d = nc.dram_tensor("x", [N, C], F32, kind="ExternalInput")
    w1_d = nc.dram_tensor("qkv_w", [O3, C], F32, kind="ExternalInput")
    b1_d = nc.dram_tensor("qkv_b", [O3], F32, kind="ExternalInput")
    w2_d = nc.dram_tensor("proj_w", [C, C], F32, kind="ExternalInput")
    b2_d = nc.dram_tensor("proj_b", [C], F32, kind="ExternalInput")
    y_d = nc.dram_tensor("y", [N, C], F32, kind="ExternalOutput")

    with tile.TileContext(nc) as tc:
        _body(nc, tc, x_d, w1_d, b1_d, w2_d, b2_d, y_d)
    nc.compile()
    return nc


def _body(nc, tc, x_d, w1_d, b1_d, w2_d, b2_d, y_d):
    from contextlib import ExitStack

    ctx = ExitStack()
    with ctx:
        const = ctx.enter_context(tc.tile_pool(name="const", bufs=1))
        aux = ctx.enter_context(tc.tile_pool(name="aux", bufs=2))
        proj = ctx.enter_context(tc.tile_pool(name="proj", bufs=3))
        anorm = ctx.enter_context(tc.tile_pool(name="anorm", bufs=4))
        scratch_cm = tc.tile_pool(name="scratch", bufs=2)
        scratch = scratch_cm.__enter__()

        # ---------------- persistent tiles ----------------
        id16 = const.tile([128, 128], F16)
        id32 = const.tile([128, 128], F32)
        cmagic = const.tile([128, 1], F32)
        b1col = [const.tile([128, 1], F32, tag=f"b1c{o}", name=f"b1c{o}") for o in range(6)]
        bqcol = [const.tile([128, 1], F32, tag=f"bqc{o}", name=f"bqc{o}") for o in range(3)]
        bv_b = const.tile([128, C], F32)
        b2_b = const.tile([128, C], F32)
        mc1_128 = const.tile([128, 1], F32)
        mc1_1024 = const.tile([128, 1], F32)
        mc2_128 = const.tile([128, 1], F32)
        sw1 = const.tile([128, 1], F32)
        sw2 = const.tile([128, 1], F32)
        amax = const.tile([128, NT], F32)
        s_st = const.tile([128, NT], F32)
        ivv = const.tile([128, NT], F32)
        row_bcast = const.tile([128, N], F16)
        t1T = [const.tile([128, O3], F16, tag=f"t1T{c}", name=f"t1T{c}") for c in range(CB)]
        t2T = [const.tile([128, C], F16, tag=f"t2T{c}", name=f"t2T{c}") for c in range(CB)]
        mT = [[const.tile([128, HN], F16, tag=f"mT{c}_{hf}", name=f"mT{c}_{hf}")
               for hf in range(2)] for c in range(CB)]
        qdup = [const.tile([128, N], F16, tag=f"qd{h}", name=f"qd{h}") for h in range(H)]
        kdup = [const.tile([128, N], F16, tag=f"kd{h}", name=f"kd{h}") for h in range(H)]
        va = [const.tile([128, H, D + 1], F16, tag=f"va{j}", name=f"va{j}") for j in range(NT)]
        stage = [const.tile([65, N], F16, tag=f"st{h}", name=f"st{h}") for h in range(H)]
        onat = [const.tile([128, C], F16, tag=f"on{j}", name=f"on{j}") for j in range(NT)]
        pmax = [const.tile([128, 1], F32, tag=f"pm{j}", name=f"pm{j}") for j in range(NT)]
        m2t = [const.tile([128, C], F16, tag=f"m2_{j}", name=f"m2_{j}") for j in range(NT)]
        iv2t = [const.tile([128, 1], F32, tag=f"iv2_{j}", name=f"iv2_{j}") for j in range(NT)]

        # ---------------- DMAs: x chunk 0, weights, then x chunks 1-3 -------
        x_all = []

        def x_dma(c):
            xc = aux.tile([128, TPC, C], F32, tag="x_all", name=f"x{c}", bufs=2)
            nc.sync.dma_start(
                xc[:],
                x_d[c * CW:(c + 1) * CW, :].rearrange("(t p) c -> p t c", p=128),
            )
            x_all.append(xc)

        x_dma(0)
        w1s = aux.tile([128, OT, C], F32, tag="w1s", name="w1s", bufs=1)
        nc.sync.dma_start(w1s[:], w1_d[:, :].rearrange("(o p) c -> p o c", p=128))
        for o in range(6):
            nc.sync.dma_start(
                b1col[o][:, 0:1],
                b1_d[o * 128:(o + 1) * 128].rearrange("(p one) -> p one", one=1),
            )
        x_dma(1)
        w2s = aux.tile([128, CB, C], F32, tag="w2s", name="w2s", bufs=1)
        nc.sync.dma_start(w2s[:], w2_d[:, :].rearrange("(o p) c -> p o c", p=128))
        bv_row = scratch.tile([1, C], F32, tag="bvr", bufs=1)
        nc.sync.dma_start(bv_row[:], b1_d[2 * C:3 * C].rearrange("(one f) -> one f", one=1))
        b2_row = scratch.tile([1, C], F32, tag="b2r", bufs=1)
        nc.sync.dma_start(b2_row[:], b2_d[:].rearrange("(one f) -> one f", one=1))
        x_dma(2)
        x_dma(3)

        make_identity(nc, id16[:])
        make_identity(nc, id32[:])
        nc.gpsimd.memset(cmagic[:], MAGIC)
        cmagic_b = cmagic[:, 0:1].broadcast_to([128, C])
        for j in range(NT):
            nc.gpsimd.memset(va[j][:], 1.0)
        nc.gpsimd.partition_broadcast(bv_b[:], bv_row[:])
        nc.gpsimd.partition_broadcast(b2_b[:], b2_row[:])

        pre16_cm = tc.tile_pool(name="pre16", bufs=3, space="PSUM")
        pre16 = pre16_cm.__enter__()
        pre32_cm = tc.tile_pool(name="pre32", bufs=3, space="PSUM")
        pre32 = pre32_cm.__enter__()

        # ---------------- weight quant helpers ----------------
        def w_reduce(ws, n_tiles, nelem, mc, sw, tag, pool):
            wsum = pool.tile([128, n_tiles], F32, tag=f"wsum{tag}", bufs=1)
            for o in range(n_tiles):
                nc.vector.tensor_reduce(
                    wsum[:, o:o + 1], ws[:, o, :], mybir.AxisListType.X, ALU.add,
                    apply_absolute_value=True,
                )
            acc = pool.tile([128, 1], F32, tag=f"acc{tag}", bufs=1)
            nc.vector.tensor_reduce(acc[:], wsum[:], mybir.AxisListType.X, ALU.add)
            allsum = pool.tile([128, 1], F32, tag=f"alls{tag}", bufs=1)
            nc.gpsimd.partition_all_reduce(allsum[:], acc[:], 128, ReduceOp.add)
            nc.vector.tensor_scalar(
                out=mc[:], in0=allsum[:], scalar1=1.0 / nelem, scalar2=1e-5,
                op0=ALU.mult, op1=ALU.max,
            )
            nc.vector.reciprocal(sw[:], mc[:])

        def w_tile(ws, o, sw, tT, tag):
            q1 = scratch.tile([128, C], F32, tag=f"q1{tag}", bufs=1)
            nc.scalar.activation(q1[:], ws[:, o, :], AF.Copy, bias=MAGIC, scale=sw[:])
            q2 = scratch.tile([128, C], F32, tag=f"q2{tag}", bufs=1)
            nc.vector.tensor_scalar(
                out=q2[:], in0=q1[:], scalar1=MAGIC, scalar2=1.0,
                op0=ALU.subtract, op1=ALU.min,
            )
            t = scratch.tile([128, C], F16, tag=f"t{tag}", name=f"t{tag}{o}", bufs=2)
            nc.vector.tensor_scalar(
                out=t[:], in0=q2[:], scalar1=-1.0, scalar2=None, op0=ALU.max,
            )
            for cb in range(CB):
                p = pre16.tile([128, 128], F16, tag="tr")
                nc.tensor.transpose(p[:], t[:, cb * 128:(cb + 1) * 128], id16[:])
                nc.scalar.copy(tT[cb][:, o * 128:(o + 1) * 128], p[:])

        def w_tile_e(ws, o, sw, tT, p16):
            """all-DVE weight quant tile for use inside the attention loop"""
            wq1 = aux.tile([128, C], F32, tag="qktmp", name="wq1", bufs=2)
            nc.vector.scalar_tensor_tensor(
                wq1[:], ws[:, o, :], sw[:], cmagic_b, ALU.mult, ALU.add)
            wq2 = aux.tile([128, C], F32, tag="qktmp", name="wq2", bufs=2)
            nc.vector.tensor_scalar(
                out=wq2[:], in0=wq1[:], scalar1=MAGIC, scalar2=1.0,
                op0=ALU.subtract, op1=ALU.min,
            )
            t = aux.tile([128, C], F16, tag="qkmid", name="wt", bufs=2)
            nc.vector.tensor_scalar(
                out=t[:], in0=wq2[:], scalar1=-1.0, scalar2=None, op0=ALU.max,
            )
            for cb in range(CB):
                p = p16.tile([128, 128], F16, tag="tr")
                nc.tensor.transpose(p[:], t[:, cb * 128:(cb + 1) * 128], id16[:])
                nc.vector.tensor_copy(tT[cb][:, o * 128:(o + 1) * 128], p[:])

        # ---------------- x quant helpers ----------------
        def x_amax(c):
            asl = slice(c * TPC, (c + 1) * TPC)
            nc.vector.tensor_reduce(
                amax[:, asl], x_all[c][:], mybir.AxisListType.X, ALU.max,
                apply_absolute_value=True,
            )
            nc.vector.tensor_scalar(out=amax[:, asl], in0=amax[:, asl],
                                    scalar1=1e-5, scalar2=None, op0=ALU.max)
            nc.vector.reciprocal(s_st[:, asl], amax[:, asl])
            nc.vector.tensor_scalar(out=s_st[:, asl], in0=s_st[:, asl],
                                    scalar1=128.0, scalar2=None, op0=ALU.mult)

        def ivv_chunk(c):
            asl = slice(c * TPC, (c + 1) * TPC)
            nc.vector.tensor_tensor(ivv[:, asl], amax[:, asl],
                                    mc1_128[:].broadcast_to([128, TPC]), ALU.mult)

        def x_tile(c, t, p16, p32, on_act=False):
            """quantize token tile t of chunk c, transpose into mT, row-bcast amax"""
            jg = c * TPC + t
            hf, tbase = c // 2, (c % 2) * TPC
            tmp32 = aux.tile([128, C], F32, tag="xq32", bufs=2)
            if on_act:
                nc.scalar.activation(tmp32[:], x_all[c][:, t, :], AF.Copy,
                                     bias=MAGIC, scale=s_st[:, jg:jg + 1])
            else:
                nc.vector.scalar_tensor_tensor(
                    tmp32[:], x_all[c][:, t, :], s_st[:, jg:jg + 1], cmagic_b,
                    ALU.mult, ALU.add)
            m16 = aux.tile([128, C], F16, tag="m16", bufs=3)
            nc.vector.tensor_scalar(out=m16[:], in0=tmp32[:], scalar1=MAGIC,
                                    scalar2=127.0, op0=ALU.subtract, op1=ALU.min)
            for cb in range(CB):
                p = p16.tile([128, 128], F16, tag="tr")
                nc.tensor.transpose(p[:], m16[:, cb * 128:(cb + 1) * 128], id16[:])
                dstc = (tbase + t) * 128
                if on_act or cb == 2:
                    nc.scalar.copy(mT[cb][hf][:, dstc:dstc + 128], p[:])
                else:
                    nc.vector.tensor_copy(mT[cb][hf][:, dstc:dstc + 128], p[:])
            pt = p32.tile([128, 128], F32, tag="mm", name="pt_amax")
            nc.tensor.transpose(pt[0:1, 0:128], amax[:, jg:jg + 1], id32[:])
            arow = aux.tile([1, 128], F16, tag="arow", bufs=2)
            if on_act:
                nc.scalar.copy(arow[:], pt[0:1, 0:128])
            else:
                nc.vector.tensor_copy(arow[:], pt[0:1, 0:128])
            nc.gpsimd.partition_broadcast(
                row_bcast[:, jg * 128:(jg + 1) * 128], arow[:])

        # ---------------- qk / v unit builders ----------------
        def qk_unit(o, c, p32, on_act):
            """compute q/k row-tile o for token chunk c (512 tokens)."""
            mc = mc1_1024 if o < 3 else mc1_128
            bias = bqcol[o] if o < 3 else b1col[o]
            dst = qdup if o < 3 else kdup
            hA, hB = 2 * (o % 3), 2 * (o % 3) + 1
            hf, mcol = c // 2, (c % 2) * 512
            cr = slice(c * 512, (c + 1) * 512)
            ps = p32.tile([128, 512], F32, tag="mm", name="psqk")
            for cb in range(CB):
                nc.tensor.matmul(
                    ps[:], t1T[cb][:, o * 128:(o + 1) * 128],
                    mT[cb][hf][:, mcol:mcol + 512],
                    start=(cb == 0), stop=(cb == CB - 1),
                )
            tmp = aux.tile([128, 512], F32, tag="qktmp", bufs=2)
            nc.vector.scalar_tensor_tensor(
                tmp[:], ps[:], mc[:], row_bcast[:, cr], ALU.mult, ALU.mult)
            mid = aux.tile([128, 512], F16, tag="qkmid", bufs=2)
            if on_act:
                nc.scalar.activation(mid[:], tmp[:], AF.Identity, bias=bias[:], scale=1.0)
            else:
                nc.vector.tensor_scalar(out=mid[:], in0=tmp[:],
                                        scalar1=bias[:], scalar2=None, op0=ALU.add)
            nc.sync.dma_start(dst[hA][0:64, cr], mid[0:64, :])
            nc.sync.dma_start(dst[hA][64:128, cr], mid[0:64, :])
            nc.sync.dma_start(dst[hB][0:64, cr], mid[64:128, :])
            nc.sync.dma_start(dst[hB][64:128, cr], mid[64:128, :])

        def v_unit(j, p32):
            psv = p32.tile([128, C], F32, tag="mm", name="psv")
            for cb in range(CB):
                nc.tensor.matmul(
                    psv[:], mT[cb][j // 8][:, (j % 8) * 128:(j % 8 + 1) * 128],
                    t1T[cb][:, 2 * C:3 * C],
                    start=(cb == 0), stop=(cb == CB - 1),
                )
            nc.vector.scalar_tensor_tensor(
                va[j][:, 0:H, 0:D],
                psv[:].rearrange("p (h d) -> p h d", h=H),
                ivv[:, j:j + 1],
                bv_b[:].rearrange("p (h d) -> p h d", h=H),
                ALU.mult, ALU.add)

        # ---------------- preamble spine (critical path to first exp) ------
        x_amax(0)
        w_reduce(w1s, OT, O3 * C, mc1_128, sw1, "1", scratch)
        nc.vector.tensor_scalar(out=mc1_1024[:], in0=mc1_128[:], scalar1=1.0 / 1024.0,
                                scalar2=None, op0=ALU.mult)
        nc.vector.tensor_scalar(out=mc1_128[:], in0=mc1_128[:], scalar1=1.0 / 128.0,
                                scalar2=None, op0=ALU.mult)
        for o in range(3):
            nc.vector.tensor_scalar(out=bqcol[o][:], in0=b1col[o][:],
                                    scalar1=0.125, scalar2=None, op0=ALU.mult)
        ivv_chunk(0)
        w_tile(w1s, 3, sw1, t1T, "1")
        w_tile(w1s, 0, sw1, t1T, "1")
        for t in range(TPC):
            x_tile(0, t, pre16, pre32, on_act=(t % 2 == 1))
        qk_unit(3, 0, pre32, on_act=True)
        qk_unit(0, 0, pre32, on_act=True)
        x_amax(1)
        ivv_chunk(1)
        for t in range(TPC):
            x_tile(1, t, pre16, pre32, on_act=(t % 2 == 0))
        qk_unit(0, 1, pre32, on_act=True)
        qk_unit(3, 1, pre32, on_act=True)
        for o in (6, 7, 8):
            w_tile(w1s, o, sw1, t1T, "1")
        for j in range(8):
            v_unit(j, pre32)
        x_amax(2)
        ivv_chunk(2)
        x_amax(3)
        ivv_chunk(3)

        pre32_cm.__exit__(None, None, None)
        pre16_cm.__exit__(None, None, None)
        scratch_cm.__exit__(None, None, None)

        # ---------------- attention (+ absorbed work) ----------------
        with (
            tc.tile_pool(name="ps_S", bufs=2, space="PSUM") as ps_S,
            tc.tile_pool(name="ps_O", bufs=1, space="PSUM") as ps_O,
            tc.tile_pool(name="att16", bufs=1, space="PSUM") as att16,
            tc.tile_pool(name="att32", bufs=1, space="PSUM") as att32,
            tc.tile_pool(name="attn", bufs=4) as attn_pool,
        ):
            elastic = []
            for t in range(TPC):
                elastic.append(lambda t=t: x_tile(2, t, att16, att32))
            elastic.append(lambda: qk_unit(3, 2, att32, on_act=False))
            elastic.append(lambda: v_unit(8, att32))
            for t in range(TPC):
                elastic.append(lambda t=t: x_tile(3, t, att16, att32))
            elastic.append(lambda: qk_unit(3, 3, att32, on_act=False))
            for j in (9, 10, 11):
                elastic.append(lambda j=j: v_unit(j, att32))
            elastic.append(lambda: qk_unit(0, 2, att32, on_act=False))
            elastic.append(lambda: qk_unit(0, 3, att32, on_act=False))
            for j in range(12, NT):
                elastic.append(lambda j=j: v_unit(j, att32))
            for o in (4, 1, 5, 2):
                elastic.append(lambda o=o: w_tile_e(w1s, o, sw1, t1T, att16))
                for c in range(NCH):
                    elastic.append(lambda o=o, c=c: qk_unit(o, c, att32, on_act=False))
            elastic.append(lambda: w_reduce(w2s, CB, C * C, mc2_128, sw2, "2", aux))
            elastic.append(lambda: nc.vector.tensor_scalar(
                out=mc2_128[:], in0=mc2_128[:], scalar1=1.0 / 128.0,
                scalar2=None, op0=ALU.mult))
            for o in range(CB):
                elastic.append(lambda o=o: w_tile_e(w2s, o, sw2, t2T, att16))

            def pump():
                if elastic:
                    elastic.pop(0)()

            def s_half(h, mi, base):
                ps = ps_S.tile([128, HN], F32, tag="S", name="S")
                ksl = slice(mi * 128, (mi + 1) * 128)
                nc.tensor.matmul(ps[:, 0:512], kdup[h][0:64, ksl],
                                 qdup[h][0:64, base:base + 512],
                                 start=True, stop=True, tile_position=(0, 0))
                nc.tensor.matmul(ps[:, 512:1024], kdup[h][64:128, ksl],
                                 qdup[h][64:128, base + 512:base + 1024],
                                 start=True, stop=True, tile_position=(64, 0))
                return ps

            pending = []

            def drain(k, p16=att16, on_act=False):
                for _ in range(min(k, len(pending))):
                    h_, j_ = pending.pop(0)
                    p = p16.tile([128, 128], F16, tag="tr", name="dtr")
                    nc.tensor.transpose(
                        p[0:128, 0:65], stage[h_][0:65, j_ * 128:(j_ + 1) * 128],
                        id16[0:65, 0:65]
                    )
                    rs = anorm.tile([128, 1], F32, tag="rs", name="rs")
                    nc.vector.reciprocal(rs[:], p[:, 64:65])
                    if on_act:
                        nc.scalar.activation(onat[j_][:, h_ * D:(h_ + 1) * D],
                                             p[:, 0:64], AF.Copy, bias=0.0, scale=rs[:])
                    else:
                        nc.vector.tensor_scalar(
                            out=onat[j_][:, h_ * D:(h_ + 1) * D], in0=p[:, 0:64],
                            scalar1=rs[:], scalar2=None, op0=ALU.mult,
                        )
                    if h_ == 0:
                        nc.vector.tensor_reduce(
                            pmax[j_][:], onat[j_][:, h_ * D:(h_ + 1) * D],
                            mybir.AxisListType.X, ALU.max, apply_absolute_value=True)
                    else:
                        hm = anorm.tile([128, 1], F32, tag="hm", name="hm")
                        nc.vector.tensor_reduce(
                            hm[:], onat[j_][:, h_ * D:(h_ + 1) * D],
                            mybir.AxisListType.X, ALU.max, apply_absolute_value=True)
                        nc.vector.tensor_tensor(pmax[j_][:], pmax[j_][:], hm[:], ALU.max)

            def tail_quant(j, on_act=False):
                """proj input act-quant for token chunk j (no PSUM use)"""
                a2c = proj.tile([128, 1], F32, tag="a2c", name="a2c")
                nc.vector.tensor_scalar(out=a2c[:], in0=pmax[j][:], scalar1=1e-5,
                                        scalar2=None, op0=ALU.max)
                s2 = proj.tile([128, 1], F32, tag="s2", name="s2")
                nc.vector.reciprocal(s2[:], a2c[:])
                nc.vector.tensor_scalar(out=s2[:], in0=s2[:], scalar1=128.0,
                                        scalar2=None, op0=ALU.mult)
                nc.vector.tensor_tensor(iv2t[j][:], a2c[:], mc2_128[:], ALU.mult)
                mq1 = proj.tile([128, C], F32, tag="pq1", name="pq1")
                if on_act:
                    nc.scalar.activation(mq1[:], onat[j][:], AF.Copy,
                                         bias=MAGIC, scale=s2[:])
                else:
                    nc.vector.scalar_tensor_tensor(
                        mq1[:], onat[j][:], s2[:], cmagic_b, ALU.mult, ALU.add)
                nc.vector.tensor_scalar(out=m2t[j][:], in0=mq1[:], scalar1=MAGIC,
                                        scalar2=127.0, op0=ALU.subtract, op1=ALU.min)

            it = 0
            for h in range(H):
                for nqh in range(2):
                    base = nqh * HN
                    pso = ps_O.tile([65, HN], F32, tag="O", name="O")
                    s = s_half(h, 0, base)
                    for mi in range(NT):
                        at = attn_pool.tile([128, HN], F16, tag="at", name="at")
                        nc.scalar.activation(at[:], s[:], AF.Exp)
                        if mi + 1 < NT:
                            s = s_half(h, mi + 1, base)
                        it += 1
                        for nq in range(2):
                            nc.tensor.matmul(
                                pso[:, nq * 512:(nq + 1) * 512],
                                va[mi][:, h, :],
                                at[:, nq * 512:(nq + 1) * 512],
                                start=(mi == 0), stop=(mi == NT - 1),
                            )
                        # absorbed work after the critical matmuls
                        if h == 0 and nqh == 0:
                            pump()
                            if mi < 6:
                                pump()
                        elif elastic:
                            if it % 3 == 0:
                                pump()
                            elif mi < 14:
                                drain(1)
                        elif h == 5 and nqh == 1:
                            # last head second half: drain h5 j<8 + quant j<8
                            if mi % 2 == 0:
                                drain(1)
                            elif mi >= 3:
                                tail_quant((mi - 3) // 2)
                        elif mi < 14:
                            drain(1)
                    # split the stage copy so the next half's first A@V
                    # (WAR on pso) only waits ~650ns, and ACT fills its bubble
                    nc.vector.tensor_copy(stage[h][0:65, base:base + 512], pso[:, 0:512])
                    nc.scalar.copy(stage[h][0:65, base + 512:base + HN], pso[:, 512:HN])
                    pending.extend((h, j) for j in range(nqh * 8, nqh * 8 + 8))

        # ---------------- proj tail ----------------
        with (
            tc.tile_pool(name="post16", bufs=4, space="PSUM") as post16,
            tc.tile_pool(name="post32", bufs=3, space="PSUM") as post32,
            tc.tile_pool(name="postsb", bufs=4) as postsb,
        ):
            def tail_pieces(j):
                pieces = []
                for cb in range(CB):
                    p = post16.tile([128, 128], F16, tag="tr", name="ptr2")
                    nc.tensor.transpose(p[:], m2t[j][:, cb * 128:(cb + 1) * 128], id16[:])
                    piece = postsb.tile([128, 128], F16, tag="piece", name="piece",
                                        bufs=8)
                    if cb == 1:
                        nc.vector.tensor_copy(piece[:], p[:])
                    else:
                        nc.scalar.copy(piece[:], p[:])
                    pieces.append(piece)
                return pieces

            def tail_mm(j, pieces):
                psf = post32.tile([128, C], F32, tag="mm", name="psf")
                for cb in range(CB):
                    nc.tensor.matmul(psf[:], pieces[cb][:], t2T[cb][:, 0:C],
                                     start=(cb == 0), stop=(cb == CB - 1))
                yt = postsb.tile([128, C], F32, tag="y", name="y")
                nc.vector.scalar_tensor_tensor(
                    yt[:], psf[:], iv2t[j][:], b2_b[:], ALU.mult, ALU.add)
                nc.sync.dma_start(y_d[j * 128:(j + 1) * 128, :], yt[:])

            # software-pipelined: proj j's transposes/copies overlap the
            # matmuls of proj j-1; h5 upper-half drains and quants weave in
            pipe = []
            for k in range(NT):
                if k < 7:
                    drain(1, p16=post16, on_act=True)
                elif pending:
                    drain(1, p16=post16, on_act=(k % 2 == 0))
                if 7 + k < NT:
                    tail_quant(7 + k, on_act=True)
                pipe.append((k, tail_pieces(k)))
                if len(pipe) > 1:
                    jj, pcs = pipe.pop(0)
                    tail_mm(jj, pcs)
            drain(len(pending), p16=post16, on_act=True)
            while pipe:
                jj, pcs = pipe.pop(0)
                tail_mm(jj, pcs)


_CACHE = {}


def _get_program():
    if "nc" not in _CACHE:
        _CACHE["nc"] = build_program()
    return _CACHE["nc"]


def kernel(x, qkv_w, qkv_b, proj_w, proj_b):
    x = np.ascontiguousarray(np.asarray(x, dtype=np.float32))
    qkv_w = np.ascontiguousarray(np.asarray(qkv_w, dtype=np.float32))
    qkv_b = np.ascontiguousarray(np.asarray(qkv_b, dtype=np.float32))
    proj_w = np.ascontiguousarray(np.asarray(proj_w, dtype=np.float32))
    proj_b = np.ascontiguousarray(np.asarray(proj_b, dtype=np.float32))

    nc = _get_program()
    in_maps = [
        {"x": x[b], "qkv_w": qkv_w, "qkv_b": qkv_b, "proj_w": proj_w, "proj_b": proj_b}
        for b in range(B)
    ]
    res = bass_utils.run_bass_kernel_spmd(nc, in_maps, core_ids=list(range(B)))
    out = np.stack([res.results[b]["y"] for b in range(B)], axis=0)
    _CACHE["last_results"] = res
    return out
